# revision 55
# baseline (speedup 1.0000x reference)
"""Trainium2 Bass kernel for nn_AdaptedTransformerBlock (dense transformer block,
cross-attention + FFN) running SPMD on 8 NeuronCores.

Sharding (data parallel, zero collectives): core c handles batch c//2 and
query rows (c%2)*512..+512. K/V for a batch's context are computed
redundantly on the pair of cores sharing that batch. Weights are cast to
bf16 and activations pre-transposed on the host; the device runs bf16
matmuls with f32 PSUM accumulation.

v2 restructure vs baseline:
- K/V projection matmuls are interleaved into the per-head-pair attention
  loop so the tensor engine stays busy while ScalarE computes the softmax
  exps (previously the attention phase was exp-bound at ~430ns/matmul).
- Softmax exp batched to N=1024 (one activation per two score chunks).
- Score matmuls (K=64) row-packed: the even head streams through PE rows
  0-63 while the odd head uses rows 64-127, concurrently.
- The 16 serial per-head reciprocals are batched into two [8,512] calls.
- The FFN second matmul produces Y^T feature-major (bias+residual fused on
  the partition axis); the host transposes the output. This removes the 32
  PE transposes and the broadcast of b2.
"""

import os

import numpy as np
import ml_dtypes


import concourse.bass as bass
import concourse.mybir as mybir
import concourse.tile as tile
from concourse import bacc

F32 = mybir.dt.float32
BF16 = mybir.dt.bfloat16
F8 = mybir.dt.float8e4
DR = mybir.MatmulPerfMode.DoubleRow
AF = mybir.ActivationFunctionType
ALU = mybir.AluOpType

P = 128
DIM = 1024
INNER = 1024
HEADS = 16
DH = 64
FF = 4096
LQ = 512          # queries per core
LK = 2048
KO = DIM // P     # 8 contraction subtiles over dim
IC = INNER // P   # 8 inner chunks
KC = LK // P      # 16 key-position chunks
FC = FF // P      # 32 ffn chunks
SCALE = 0.125     # 1/sqrt(64)
EXP_BIAS = -5.545177444479562  # -ln(256): keeps exp'd scores in fp8 range;
                               # cancels in the softmax normalization
DEBUG_TAPS = bool(int(os.environ.get("KERNEL_DEBUG_TAPS", "0")))
# Bisect toggles (default = fast path)
OPT_ROWPACK = bool(int(os.environ.get("KERNEL_ROWPACK", "1")))
OPT_EXPBATCH = bool(int(os.environ.get("KERNEL_EXPBATCH", "1")))
OPT_AVINTER = bool(int(os.environ.get("KERNEL_AVINTER", "1")))
OPT_PSO65 = bool(int(os.environ.get("KERNEL_PSO65", "1")))


def build(nc: bass.Bass):
    # ---- kernel I/O -------------------------------------------------------
    xT_ext = nc.dram_tensor("xT", [DIM, LQ], BF16, kind="ExternalInput")
    x8_ext = nc.dram_tensor("x8", [P, 4, 2, LQ], F8, kind="ExternalInput")
    ctx8_ext = nc.dram_tensor("ctx8", [P, 4, 2, LK], F8, kind="ExternalInput")
    wq8_ext = nc.dram_tensor("Wq8", [P, 4, 2, INNER], F8, kind="ExternalInput")
    wk8_ext = nc.dram_tensor("Wk8", [P, 4, 2, INNER], F8, kind="ExternalInput")
    wv8_ext = nc.dram_tensor("Wv8", [P, 4, 2, INNER], F8, kind="ExternalInput")
    wo8_ext = nc.dram_tensor("Wo8", [P, 4, 2, DIM], F8, kind="ExternalInput")
    bo_ext = nc.dram_tensor("bo", [DIM], F32, kind="ExternalInput")
    w1_ext = nc.dram_tensor("W1", [DIM, FF], BF16, kind="ExternalInput")
    b1_ext = nc.dram_tensor("b1", [FF], F32, kind="ExternalInput")
    w2_ext = nc.dram_tensor("W2", [FF, DIM], BF16, kind="ExternalInput")
    b2_ext = nc.dram_tensor("b2", [DIM], F32, kind="ExternalInput")
    out_ext = nc.dram_tensor("out", [DIM, LQ], F32, kind="ExternalOutput")

    xT_t = xT_ext[:].rearrange("(ko p) q -> p ko q", p=P)      # [128,8,512]
    w1_t = w1_ext[:].rearrange("(ko p) f -> p ko f", p=P)      # [128,8,4096]
    w2_t = w2_ext[:].rearrange("(ks p) d -> p ks d", p=P)      # [128,32,1024]
    bo_t = bo_ext[:].rearrange("(c p) -> p c", p=P)            # [128,8]
    b1_t = b1_ext[:].rearrange("(c p) -> p c", p=P)            # [128,32]
    b2_t = b2_ext[:].rearrange("(c p) -> p c", p=P)            # [128,8]
    out_t = out_ext[:].rearrange("(dc p) q -> p dc q", p=P)    # [128,8,512]

    from contextlib import ExitStack

    with tile.TileContext(nc) as tc:
        with ExitStack() as stk:
            ec = stk.enter_context
            const = ec(tc.tile_pool(name="const", bufs=1))
            dram = ec(tc.tile_pool(name="dram", bufs=1, space="DRAM"))
            pA = ec(tc.tile_pool(name="pA", bufs=1))        # ctxT, later HT
            pKT = ec(tc.tile_pool(name="pKT", bufs=1))
            pV = ec(tc.tile_pool(name="pV", bufs=1))
            pQT = ec(tc.tile_pool(name="pQT", bufs=1))      # QT, later X1Tb
            pXT = ec(tc.tile_pool(name="pXT", bufs=1))
            pOT = ec(tc.tile_pool(name="pOT", bufs=1))
            pPT = ec(tc.tile_pool(name="pPT",
                                  bufs=(10 if OPT_AVINTER else 18)))
            wres = ec(tc.tile_pool(name="wres", bufs=3))    # resident weights
            wstr = ec(tc.tile_pool(name="wstr", bufs=3))    # streamed W1/W2
            io32 = ec(tc.tile_pool(name="io32", bufs=2))
            rp = ec(tc.tile_pool(name="rp", bufs=2))
            smallp = ec(tc.tile_pool(name="smallp", bufs=1))
            psmm = ec(tc.tile_pool(name="psmm", bufs=2, space="PSUM"))
            pss = ec(tc.tile_pool(name="pss", bufs=2, space="PSUM"))
            pso = ec(tc.tile_pool(name="pso", bufs=2, space="PSUM"))
            r_dram = dram.tile([HEADS, LQ], F32)

            # ---- activations + resident weights (priority order) ---------
            # fp8 (DoubleRow) copies of x/ctx/Wq/Wk/Wv: contraction dim is
            # pre-paired on the host as [p, c2, j, n] with dim = c2*256 +
            # j*128 + p, so each matmul contracts 256 dims.
            xT = pXT.tile([P, KO, LQ], BF16, tag="xtok")       # x^T [dim, q]
            x8 = pXT.tile([P, 4, 2, LQ], F8, tag="x8")
            wq8 = wres.tile([P, 4, 2, INNER], F8, tag="w8", name="wq8", bufs=3)
            nc.sync.dma_start(x8[:], x8_ext[:])
            nc.sync.dma_start(wq8[:], wq8_ext[:])
            nc.sync.dma_start(xT[:], xT_t)

            boT = const.tile([P, KO], F32, tag="boT")
            nc.sync.dma_start(boT[:], bo_t)
            b1T = const.tile([P, FC], F32, tag="b1T")
            nc.sync.dma_start(b1T[:], b1_t)
            b2T = const.tile([P, KO], F32, tag="b2T")
            nc.sync.dma_start(b2T[:], b2_t)
            ebias = const.tile([P, 1], F32, tag="ebias")
            nc.any.memset(ebias[:], EXP_BIAS)

            ctx8 = pA.tile([P, 4, 2, LK], F8, tag="a4")
            wk8 = wres.tile([P, 4, 2, INNER], F8, tag="w8", name="wk8", bufs=3)
            wv8 = wres.tile([P, 4, 2, INNER], F8, tag="w8", name="wv8", bufs=3)
            nc.sync.dma_start(wk8[:], wk8_ext[:])
            nc.sync.dma_start(ctx8[:, :, :, 0:512], ctx8_ext[:][:, :, :, 0:512])
            nc.sync.dma_start(wv8[:], wv8_ext[:])
            for k4 in range(1, 4):
                nc.sync.dma_start(
                    ctx8[:, :, :, k4 * 512:(k4 + 1) * 512],
                    ctx8_ext[:][:, :, :, k4 * 512:(k4 + 1) * 512])

            def mm1():
                return psmm.tile([P, 512], F32, tag="mm", name="mmps")

            # ---- Q^T = Wq^T @ x^T  (feature-major, fp8 DoubleRow) ---------
            QT = pQT.tile([P, IC, LQ], BF16, tag="qt")
            with nc.named_scope("qproj"):
                for ic in range(IC):
                    ps = mm1()
                    for c2 in range(4):
                        nc.tensor.matmul(
                            ps[:], wq8[:, c2, :, ic * P:(ic + 1) * P],
                            x8[:, c2, :, :],
                            start=(c2 == 0), stop=(c2 == 3), perf_mode=DR)
                    nc.vector.tensor_copy(out=QT[:, ic, :], in_=ps[:])

            wo8 = wres.tile([P, 4, 2, DIM], F8, tag="w", name="wo8", bufs=1)
            nc.sync.dma_start(wo8[:], wo8_ext[:])

            # ---- K^T / V emission units (interleaved with attention) ------
            # KT rotates per inner chunk: pair ic's slice dies right after
            # its scores, so 3 live buffers suffice (saves 20KB/partition).
            kt_tiles = {}

            def kt_tile(ic):
                if ic not in kt_tiles:
                    kt_tiles[ic] = pKT.tile([P, LK], BF16, tag="kt",
                                            name="kt", bufs=3)
                return kt_tiles[ic]

            V4 = pV.tile([P, KC, HEADS, DH + 1], F8, tag="v4")
            nc.any.memset(V4[:, :, :, DH:DH + 1], 1.0)

            def k_unit(ic, k4):
                """K^T cols for inner chunk ic, key block k4 (512 wide)."""
                kt = kt_tile(ic)
                ps = mm1()
                for c2 in range(4):
                    nc.tensor.matmul(
                        ps[:], wk8[:, c2, :, ic * P:(ic + 1) * P],
                        ctx8[:, c2, :, k4 * 512:(k4 + 1) * 512],
                        start=(c2 == 0), stop=(c2 == 3), perf_mode=DR)
                nc.any.tensor_copy(
                    out=kt[:, k4 * 512:(k4 + 1) * 512], in_=ps[:])

            def v_unit(kp, iw):
                """V token-major rows kp*128..+128, inner half iw (8 heads)."""
                ps = mm1()
                for c2 in range(4):
                    nc.tensor.matmul(
                        ps[:], ctx8[:, c2, :, kp * P:(kp + 1) * P],
                        wv8[:, c2, :, iw * 512:(iw + 1) * 512],
                        start=(c2 == 0), stop=(c2 == 3), perf_mode=DR)
                nc.any.tensor_copy(
                    out=V4[:, kp, iw * 8:(iw + 1) * 8, 0:DH],
                    in_=ps[:].rearrange("p (h d) -> p h d", d=DH))

            # Deferred K/V units in the order the attention loop consumes
            # them. V half iw is consumed inside pair 4*iw (rate-2 pumping
            # keeps ahead of that pair's attnV); K(ic) must land before pair
            # ic's scores.
            extras = (
                [("v", kp, 0) for kp in range(KC)]
                + [("k", ic, k4) for ic in range(1, 5) for k4 in range(4)]
                + [("v", kp, 1) for kp in range(KC)]
                + [("k", ic, k4) for ic in range(5, IC) for k4 in range(4)]
            )
            start_need = [0, 20, 24, 28, 32, 52, 56, 60]

            def run_unit(u):
                if u[0] == "k":
                    k_unit(u[1], u[2])
                else:
                    v_unit(u[1], u[2])

            with nc.named_scope("kv0"):
                for k4 in range(4):
                    k_unit(0, k4)

            # ---- attention: head pairs with K/V work interleaved ----------
            OT = pOT.tile([P, IC, LQ], F8, tag="ot")
            pending_muls = []

            ei = 0  # index into extras

            def pump(n):
                nonlocal ei
                for _ in range(min(n, len(extras) - ei)):
                    run_unit(extras[ei])
                    ei += 1

            with nc.named_scope("attn"):
                for hp in range(IC):
                    ic = hp
                    hA, hB = 2 * hp, 2 * hp + 1
                    pump(start_need[hp] - ei)
                    # pairs 0 and 4 pump their V half at rate 2 (stays ahead
                    # of their own attnV); other pairs trickle K units
                    rate = 2 if hp % 4 == 0 else 1

                    if OPT_PSO65:
                        ps_oA = pso.tile([DH + 1, 512], F32, tag="po",
                                         name="psoA")
                        ps_oB = pso.tile([DH + 1, 512], F32, tag="po",
                                         name="psoB")
                    else:
                        ps_oA = pso.tile([P, 512], F32, tag="po",
                                         name="psoA")[0:DH + 1, :]
                        ps_oB = pso.tile([P, 512], F32, tag="po",
                                         name="psoB")[0:DH + 1, :]

                    def score_unit(h, po, u):
                        kt = kt_tile(ic)
                        ps2 = pss.tile([P, 2, 512], F32, tag="s2", name="ps2")
                        for j in range(2):
                            kc = 2 * u + j
                            nc.tensor.matmul(
                                ps2[:, j, :],
                                kt[po:po + DH, kc * P:(kc + 1) * P],
                                QT[po:po + DH, ic, :], start=True, stop=True)
                        pt = pPT.tile([P, 2, LQ], F8, tag="pt", name="pt")
                        nc.scalar.activation(
                            out=pt[:], in_=ps2[:], func=AF.Exp, scale=SCALE,
                            bias=ebias[:])
                        return pt

                    def av_unit(h, ps_o, pt, u):
                        nc.tensor.matmul(
                            ps_o[:], V4[:, 2 * u:2 * u + 2, h, :], pt[:],
                            start=(u == 0), stop=(u == 7),
                            perf_mode=DR, skip_group_check=True)

                    if OPT_ROWPACK:
                        pts = []
                        for u in range(8):      # units of 2 key chunks
                            ptA = score_unit(hA, 0, u)
                            ptB = score_unit(hB, DH, u)
                            pts.append((ptA, ptB))
                            pump(rate if (hp % 4 == 0 or u % 2 == 1) else 0)
                            if OPT_AVINTER and u > 0:
                                pA_, pB_ = pts[u - 1]
                                av_unit(hA, ps_oA, pA_, u - 1)
                                av_unit(hB, ps_oB, pB_, u - 1)
                        pump(rate)
                        if OPT_AVINTER:
                            pA_, pB_ = pts[7]
                            av_unit(hA, ps_oA, pA_, 7)
                            av_unit(hB, ps_oB, pB_, 7)
                        else:
                            for u in range(8):
                                av_unit(hA, ps_oA, pts[u][0], u)
                            for u in range(8):
                                av_unit(hB, ps_oB, pts[u][1], u)
                    else:
                        for h, po, ps_o in ((hA, 0, ps_oA), (hB, DH, ps_oB)):
                            pts = []
                            for u in range(8):
                                pts.append(score_unit(h, po, u))
                                pump(rate if h == hA else 0)
                                if OPT_AVINTER and u > 0:
                                    av_unit(h, ps_o, pts[u - 1], u - 1)
                            if OPT_AVINTER:
                                av_unit(h, ps_o, pts[7], 7)
                            else:
                                for u in range(8):
                                    av_unit(h, ps_o, pts[u], u)
                    # Drain PSUM fast: copy unnormalized O^T plus the raw
                    # sumexp rows (cheap), then defer everything slow
                    # (broadcast roundtrip, reciprocal, multiply) one pair so
                    # it never delays freeing ps_o for the next pair. The
                    # reciprocal runs AFTER the broadcast on 128 partitions
                    # (0.5us) instead of on a single partition (3.3us).
                    nc.vector.tensor_copy(out=OT[0:DH, ic, :], in_=ps_oA[0:DH, :])
                    nc.vector.tensor_copy(out=OT[DH:P, ic, :], in_=ps_oB[0:DH, :])
                    ssA = rp.tile([1, LQ], F32, tag="rs", name="ssA")
                    ssB = rp.tile([1, LQ], F32, tag="rs", name="ssB")
                    nc.vector.tensor_copy(out=ssA[:], in_=ps_oA[DH:DH + 1, :])
                    nc.vector.tensor_copy(out=ssB[:], in_=ps_oB[DH:DH + 1, :])
                    nc.sync.dma_start(r_dram[hA:hA + 1, :], ssA[:])
                    nc.sync.dma_start(r_dram[hB:hB + 1, :], ssB[:])

                    def norm_muls(ic=ic, hA=hA, hB=hB):
                        sbc2 = rp.tile([P, LQ], F32, tag="sbc", name="sbc2")
                        nc.gpsimd.dma_start(
                            sbc2[0:DH, :],
                            r_dram[hA:hA + 1, :].to_broadcast((DH, LQ)))
                        nc.gpsimd.dma_start(
                            sbc2[DH:P, :],
                            r_dram[hB:hB + 1, :].to_broadcast((DH, LQ)))
                        nc.vector.reciprocal(out=sbc2[:], in_=sbc2[:])
                        nc.vector.tensor_mul(
                            out=OT[0:DH, ic, :], in0=OT[0:DH, ic, :],
                            in1=sbc2[0:DH, :])
                        nc.vector.tensor_mul(
                            out=OT[DH:P, ic, :], in0=OT[DH:P, ic, :],
                            in1=sbc2[DH:P, :])

                    if pending_muls:
                        pending_muls.pop()()
                    pending_muls.append(norm_muls)

            pump(len(extras))  # safety: emit anything left
            pending_muls.pop()()  # pair 7's muls (Wo's ic=7 matmuls wait)

            if DEBUG_TAPS:
                qt_dbg = nc.dram_tensor("QT_dbg", [P, IC, LQ], BF16,
                                        kind="ExternalOutput")
                v4_dbg = nc.dram_tensor("V4_dbg", [P, KC, HEADS, DH + 1], BF16,
                                        kind="ExternalOutput")
                ot_dbg = nc.dram_tensor("OT_dbg", [P, IC, LQ], BF16,
                                        kind="ExternalOutput")
                nc.sync.dma_start(qt_dbg[:], QT[:])
                nc.sync.dma_start(v4_dbg[:], V4[:])
                nc.sync.dma_start(ot_dbg[:], OT[:])

            # ---- X1^T = Wo^T @ O^T + bo + x^T  (feature-major, bf16) ------
            # ic-outer with 8 accumulators: heads 0-13's matmuls overlap the
            # last pair's normalization chain (only ic=7 waits for it).
            X1Tb = pQT.tile([P, KO, LQ], BF16, tag="qt", name="X1Tb")
            with nc.named_scope("wo"):
                wo_accs = []
                for dc in range(KO):
                    if dc < 2:
                        wo_accs.append(mm1())
                    elif dc < 6:
                        if dc % 2 == 0:
                            wops2 = pss.tile([P, 2, 512], F32, tag="s2",
                                             name="wops2")
                        wo_accs.append(wops2[:, dc % 2, :])
                    else:
                        wo_accs.append(pso.tile([P, 512], F32, tag="po",
                                                name="psoWo"))
                for c2 in range(4):
                    for dc in range(KO):
                        nc.tensor.matmul(
                            wo_accs[dc], wo8[:, c2, :, dc * P:(dc + 1) * P],
                            OT[:, 2 * c2:2 * c2 + 2, :],
                            start=(c2 == 0), stop=(c2 == 3),
                            perf_mode=DR, skip_group_check=True)
                for dc in range(KO):
                    nc.vector.scalar_tensor_tensor(
                        out=X1Tb[:, dc, :], in0=wo_accs[dc],
                        scalar=boT[:, dc:dc + 1],
                        in1=xT[:, dc, :], op0=ALU.add, op1=ALU.add)

            if DEBUG_TAPS:
                x1_dbg = nc.dram_tensor("X1_dbg", [P, KO, LQ], BF16,
                                        kind="ExternalOutput")
                nc.sync.dma_start(x1_dbg[:], X1Tb[:])

            # ---- H^T = gelu(W1^T @ X1^T + b1)  (feature-major) ------------
            HT = pA.tile([P, FC, LQ], BF16, tag="a4", name="HT")  # reuse ctxT
            with nc.named_scope("ffn1"):
                # two alternating quartets of single-bank accumulators give
                # pipeline depth 2 (next group's matmuls never wait on this
                # group's gelus)
                pssA = pss.tile([P, 2, 512], F32, tag="s2", name="pssA")
                pssB = pss.tile([P, 2, 512], F32, tag="s2", name="pssB")
                quart = [
                    [mm1(), mm1(), pssA[:, 0, :], pssA[:, 1, :]],
                    [pssB[:, 0, :], pssB[:, 1, :],
                     pso.tile([P, 512], F32, tag="po", name="psoW1a"),
                     pso.tile([P, 512], F32, tag="po", name="psoW1b")],
                ]
                for wg in range(8):             # groups of 4 f-chunks
                    accs = quart[wg % 2]
                    for ko in range(KO):
                        wb = wstr.tile([P, 512], BF16, tag="w1s", name="w1b",
                                       bufs=6)
                        nc.gpsimd.dma_start(
                            wb[:], w1_t[:, ko, wg * 512:(wg + 1) * 512])
                        for j in range(4):
                            nc.tensor.matmul(
                                accs[j], wb[:, j * P:(j + 1) * P],
                                X1Tb[:, ko, :],
                                start=(ko == 0), stop=(ko == KO - 1),
                                skip_group_check=True)
                    for j in range(4):
                        fc = 4 * wg + j
                        nc.scalar.activation(
                            out=HT[:, fc, :], in_=accs[j], func=AF.Gelu,
                            bias=b1T[:, fc:fc + 1])

            if DEBUG_TAPS:
                ht_dbg = nc.dram_tensor("HT_dbg", [P, FC, LQ], BF16,
                                        kind="ExternalOutput")
                nc.sync.dma_start(ht_dbg[:], HT[:])

            # ---- Y^T = W2^T @ H^T + b2 + X1^T  (feature-major, f32 out) ---
            with nc.named_scope("ffn2"):
                accs = []
                for dc in range(KO):
                    if dc < 2:
                        accs.append(mm1())
                    elif dc < 6:
                        if dc % 2 == 0:
                            ps2 = pss.tile([P, 2, 512], F32, tag="s2",
                                           name="ps2w2")
                        accs.append(ps2[:, dc % 2, :])
                    else:
                        accs.append(pso.tile([P, 512], F32, tag="po",
                                             name="psoY"))
                for ks in range(FC):
                    wb = wstr.tile([P, DIM], BF16, tag="w2s", name="w2b",
                                   bufs=4)
                    nc.gpsimd.dma_start(wb[:], w2_t[:, ks, :])
                    for dc in range(KO):
                        nc.tensor.matmul(
                            accs[dc], wb[:, dc * P:(dc + 1) * P], HT[:, ks, :],
                            start=(ks == 0), stop=(ks == FC - 1))
                for dc in range(KO):
                    osb = io32.tile([P, 512], F32, tag="io32", name="osb")
                    nc.vector.scalar_tensor_tensor(
                        out=osb[:], in0=accs[dc], scalar=b2T[:, dc:dc + 1],
                        in1=X1Tb[:, dc, :], op0=ALU.add, op1=ALU.add)
                    nc.sync.dma_start(out_t[:, dc, :], osb[:])

    return nc


_NC_CACHE = {}


def _get_nc():
    if "nc" not in _NC_CACHE:
        nc = bacc.Bacc("TRN2", target_bir_lowering=False, debug=False, num_devices=8)
        build(nc)
        nc.compile()
        _NC_CACHE["nc"] = nc
    return _NC_CACHE["nc"]


def run_full(inputs, trace=False):
    """Run on all 8 cores. Returns (full_output [4,1024,1024] f32, exec_time_ns)."""
    from concourse.bass_utils import run_bass_kernel_spmd

    nc = _get_nc()
    bf = ml_dtypes.bfloat16
    f8 = ml_dtypes.float8_e4m3
    f = lambda a: np.asarray(a, dtype=np.float32)
    x = f(inputs["x"]); ctx = f(inputs["context"])

    def pack8(aT):
        # [1024, N] -> [128, 4, 2, N] fp8 with dim = c2*256 + j*128 + p
        n = aT.shape[1]
        return np.ascontiguousarray(
            aT.reshape(4, 2, P, n).transpose(2, 0, 1, 3).astype(f8))

    shared = {
        "Wq8": pack8(f(inputs["Wq"])),
        "Wk8": pack8(f(inputs["Wk"])),
        "Wv8": pack8(f(inputs["Wv"])),
        "Wo8": pack8(f(inputs["Wo"])),
        "W1": np.ascontiguousarray(f(inputs["W1"]).astype(bf)),
        "W2": np.ascontiguousarray(f(inputs["W2"]).astype(bf)),
        "bo": np.ascontiguousarray(f(inputs["bo"])),
        "b1": np.ascontiguousarray(f(inputs["b1"])),
        "b2": np.ascontiguousarray(f(inputs["b2"])),
    }
    ctx8_b = [pack8(ctx[b].T) for b in range(4)]
    in_maps = []
    for c in range(8):
        b, qs = c // 2, (c % 2) * LQ
        m = dict(shared)
        xTc = x[b, qs:qs + LQ, :].T
        m["xT"] = np.ascontiguousarray(xTc.astype(bf))
        m["x8"] = pack8(xTc)
        m["ctx8"] = ctx8_b[b]
        in_maps.append(m)
    res = run_bass_kernel_spmd(nc, in_maps, core_ids=list(range(8)), trace=trace)
    out = np.empty((4, 1024, 1024), dtype=np.float32)
    for c in range(8):
        b, qs = c // 2, (c % 2) * LQ
        out[b, qs:qs + LQ, :] = res.results[c]["out"].T
    if trace:
        import json, os
        info = {"profile_json": res.profile_json,
                "scope_times": res.per_core_scope_times}
        try:
            with open("/tmp/last_run_info.json", "w") as fh:
                json.dump(info, fh)
        except OSError:
            pass
    return out, res.exec_time_ns


def kernel(**inputs):
    return run_full(inputs)[0]
